# revision 32
# baseline (speedup 1.0000x reference)
"""PositionAttention kernel for TRN2 (8 cores, data-parallel over batch).

Math (per batch row b):
  rel_dec  = (step+1)/len_b
  shared   = [mu_old, sigma_old/len, mc_old, ma_old, rel_dec]           (5)
  pos_in   = [dec(512), shared]                                        (517)
  blocks   = [shared, 1/len, -1/len, 1]                                  (8)
  mu_w     = relu(pos_in @ Wmu1 + bmu1) @ Wmu2 + bmu2                    (8)
  sg_w     = relu(pos_in @ Wsg1 + bsg1) @ Wsg2 + bsg2                    (8)
  mu       = sum(mu_w * blocks);  sigma = relu(sum(sg_w * blocks)) + .05
  conf     = sigmoid((-sigma + relu(conf_bias)) / conf_temp)
  z_i      = exp(-(((i+1)/len - mu)/sigma)^2) for i < len else 0
  attn     = z / max(sum(z), 1e-12)

Layout: 16 batch rows/core.  Big tile [128, 2048]: partition p = 8*b + j
(b local row, j L-chunk of 2048), free f -> position i = 2048*j + f.
"""

import numpy as np
import orjson

import concourse.bass as bass
import concourse.tile as tile
from concourse import mybir
from concourse.bass_utils import run_bass_kernel_spmd
from concourse.masks import make_identity

B, L, D, H, NB = 128, 16384, 512, 32, 8
NCORES = 8
BC = B // NCORES          # 16 batch rows per core
J = 8                     # L-chunks per row
CF = L // J               # 2048 free elems per chunk
P = BC * J                # 128 partitions
IN = D + 5                # 517
MIN_SIGMA = 0.05

FP = mybir.dt.float32
AF = mybir.ActivationFunctionType
ALU = mybir.AluOpType

NCHUNK = 4
CW = CF // NCHUNK         # 512 free elems per chunk
USE_DERF = True           # fused exp(-x^2) via Derivative_Erf (not in sim)


def _build_nc():
    nc = bass.Bass("TRN2")

    decT = nc.dram_tensor("decT", [D, BC], FP, kind="ExternalInput")
    w1 = nc.dram_tensor("w1", [IN, 2 * H], FP, kind="ExternalInput")
    b1 = nc.dram_tensor("b1", [2 * H, 1], FP, kind="ExternalInput")
    w2 = nc.dram_tensor("w2", [2 * H, 2 * NB], FP, kind="ExternalInput")
    b2 = nc.dram_tensor("b2", [2 * NB, 1], FP, kind="ExternalInput")
    # scl[b] = [mu_old, sg_old, mc_old, ma_old, len,
    #           step+1, -1/T, relu(cb)/T]
    scl = nc.dram_tensor("scl", [BC, 8], FP, kind="ExternalInput")

    attn = nc.dram_tensor("attn", [BC, L], FP, kind="ExternalOutput")
    mu_o = nc.dram_tensor("mu_o", [BC, 1], FP, kind="ExternalOutput")
    sg_o = nc.dram_tensor("sg_o", [BC, 1], FP, kind="ExternalOutput")
    cf_o = nc.dram_tensor("cf_o", [BC, 1], FP, kind="ExternalOutput")

    with tile.TileContext(nc) as tc:
        with (
            tc.tile_pool(name="const", bufs=1) as const,
            tc.tile_pool(name="big", bufs=1) as big,
            tc.psum_pool(name="ps", bufs=1) as psp,
        ):
            # ---- load small inputs (coalesced DMAs) ----
            sb_decT = const.tile([128, 4 * BC], FP)
            nc.sync.dma_start(
                out=sb_decT.rearrange("p (k n) -> p k n", k=4),
                in_=decT[:].rearrange("(k p) n -> p k n", k=4),
            )
            sb_w1 = const.tile([128, 4 * 2 * H], FP)
            nc.sync.dma_start(
                out=sb_w1.rearrange("p (k h) -> p k h", k=4),
                in_=w1[0:512, :].rearrange("(k p) h -> p k h", k=4),
            )
            sb_w1t = const.tile([5, 2 * H], FP)
            nc.sync.dma_start(out=sb_w1t, in_=w1[4 * 128:IN, :])
            sb_w2 = const.tile([2 * H, 2 * NB], FP)
            nc.sync.dma_start(out=sb_w2, in_=w2[:])
            sb_b1 = const.tile([2 * H, 1], FP)
            nc.sync.dma_start(out=sb_b1, in_=b1[:])
            sb_b2 = const.tile([2 * NB, 1], FP)
            nc.sync.dma_start(out=sb_b2, in_=b2[:])
            sb_scl = const.tile([BC, 8], FP)
            nc.sync.dma_start(out=sb_scl, in_=scl[:])

            # Stage all PE inputs through DVE copies: a Matmult's codegen
            # (fp32 LW expansion) overflows its sync-wait slots when waiting
            # directly on DMA-queue semaphores, so PE must only ever wait on
            # engine semaphores.
            decTc = const.tile([128, 4 * BC], FP)
            nc.vector.tensor_copy(decTc, sb_decT)
            w1c = const.tile([128, 4 * 2 * H], FP)
            nc.vector.tensor_copy(w1c, sb_w1)
            w1tc = const.tile([5, 2 * H], FP)
            nc.vector.tensor_copy(w1tc, sb_w1t)
            w2c = const.tile([2 * H, 2 * NB], FP)
            nc.vector.tensor_copy(w2c, sb_w2)

            # ---- per-b scalars, column layout [16,1] ----
            lenf = sb_scl[:, 4:5]
            invl = const.tile([BC, 1], FP)
            nc.vector.reciprocal(invl, lenf)

            # blocks matrix, column layout [16(b), 8]:
            # [mu_old, sg_old/len, mc_old, ma_old, (step+1)/len, 1/len, -1/len, 1]
            bcol = const.tile([BC, NB], FP)
            nc.vector.tensor_copy(bcol[:, 0:4], sb_scl[:, 0:4])
            nc.vector.tensor_mul(bcol[:, 1:2], sb_scl[:, 1:2], invl)
            nc.vector.tensor_scalar(
                bcol[:, 4:5], invl, sb_scl[:, 5:6], None, op0=ALU.mult)
            nc.vector.tensor_copy(bcol[:, 5:6], invl)
            nc.vector.tensor_scalar(
                bcol[:, 6:7], invl, -1.0, None, op0=ALU.mult)
            nc.vector.memset(bcol[:, 7:8], 1.0)

            # ---- constant matrices ----
            id16 = const.tile([BC, BC], FP)
            make_identity(nc, id16)
            # E8T[b,p] = 1 iff p//8 == b   (broadcast b -> 8 partitions)
            # keep where p-8b >= 0 AND 7-(p-8b) >= 0
            e8t = const.tile([BC, 128], FP)
            nc.gpsimd.memset(e8t, 1.0)
            nc.gpsimd.affine_select(
                out=e8t, in_=e8t, compare_op=ALU.is_ge, fill=0.0,
                base=0, pattern=[[1, 128]], channel_multiplier=-8)
            nc.gpsimd.affine_select(
                out=e8t, in_=e8t, compare_op=ALU.is_ge, fill=0.0,
                base=7, pattern=[[-1, 128]], channel_multiplier=8)
            # E8[p,b] = 1 iff p//8 == b   (reduce 8 partitions -> b)
            e8 = const.tile([128, BC], FP)
            nc.gpsimd.memset(e8, 1.0)
            nc.gpsimd.affine_select(
                out=e8, in_=e8, compare_op=ALU.is_ge, fill=0.0,
                base=0, pattern=[[-8, BC]], channel_multiplier=1)
            nc.gpsimd.affine_select(
                out=e8, in_=e8, compare_op=ALU.is_ge, fill=0.0,
                base=7, pattern=[[8, BC]], channel_multiplier=-1)

            # A Matmult can carry at most ONE sync wait in walrus codegen
            # (fp32 LW expansion), so every PE input must be last-written by
            # a single engine (DVE); Pool-built constants get DVE copies.
            id16c = const.tile([BC, BC], FP)
            nc.vector.tensor_copy(id16c, id16)
            e8tc = const.tile([BC, 128], FP)
            nc.vector.tensor_copy(e8tc, e8t)
            e8c = const.tile([128, BC], FP)
            nc.vector.tensor_copy(e8c, e8)

            # ACT staging: every compute instruction may carry at most ONE
            # sync wait, so ACT pre-stages its bias operands (one DMA wait
            # each) and later ACT ops wait only on their tensor producer.
            b1a = const.tile([2 * H, 1], FP)
            nc.scalar.activation(b1a, sb_b1, AF.Identity, bias=0.0, scale=1.0)
            b2a = const.tile([2 * NB, 1], FP)
            nc.scalar.activation(b2a, sb_b2, AF.Identity, bias=0.0, scale=1.0)
            scl_a = const.tile([BC, 2], FP)
            nc.scalar.activation(
                scl_a, sb_scl[:, 6:8], AF.Identity, bias=0.0, scale=1.0)
            # pre-wait: one ACT op waiting on the last ACT staging op, so
            # later ACT ops' own-engine bias deps are elided (max-count rule)
            scr_s = const.tile([1, 1], FP)
            nc.scalar.activation(
                scr_s, scl_a[0:1, 0:1], AF.Identity, bias=0.0, scale=1.0)

            # shared.T [5,16] row layout for the MLP tail matmul
            shr_ps = psp.tile([5, BC], FP)
            nc.tensor.transpose(shr_ps, bcol[:, 0:5], id16c)
            shr_sb = const.tile([5, BC], FP)
            nc.vector.tensor_copy(shr_sb, shr_ps)

            # ---- MLP: H1 = W1cat.T @ pos_inT  -> [64, 16] ----
            h1_ps = psp.tile([2 * H, BC], FP)
            for k in range(4):
                nc.tensor.matmul(
                    h1_ps,
                    lhsT=w1c[:, k * 2 * H:(k + 1) * 2 * H],
                    rhs=decTc[:, k * BC:(k + 1) * BC],
                    start=(k == 0), stop=False)
            nc.tensor.matmul(
                h1_ps, lhsT=w1tc, rhs=shr_sb, start=False, stop=True)
            a_sb = const.tile([2 * H, BC], FP)
            nc.scalar.activation(a_sb, h1_ps, AF.Relu, bias=b1a, scale=1.0)

            # WS = W2blk.T @ A -> [16,16] (rows: 8 mu weights, 8 sg weights)
            ws_ps = psp.tile([2 * NB, BC], FP)
            nc.tensor.matmul(ws_ps, lhsT=w2c, rhs=a_sb, start=True, stop=True)
            wsb = const.tile([2 * NB, BC], FP)
            nc.scalar.activation(wsb, ws_ps, AF.Identity, bias=b2a, scale=1.0)

            # transpose -> [16(b), 16(weights)]
            wt_ps = psp.tile([BC, BC], FP)
            nc.tensor.transpose(wt_ps, wsb, id16c)

            # TensorTensorReduce serializes as a raw-ISA instruction this
            # walrus build rejects; emulate with TensorTensor mult + an ACT
            # Identity pass whose accum_out does the free-dim reduction.
            bcol2 = const.tile([BC, 2 * NB], FP)
            nc.vector.tensor_copy(bcol2[:, 0:NB], bcol)
            nc.vector.tensor_copy(bcol2[:, NB:2 * NB], bcol)
            scr_d = const.tile([1, 1], FP)
            nc.vector.tensor_copy(scr_d, wt_ps[0:1, 0:1])  # pre-wait on PE
            prod = const.tile([BC, 2 * NB], FP)
            nc.vector.tensor_mul(prod, wt_ps, bcol2)
            junk = const.tile([BC, NB], FP)
            mu16 = const.tile([BC, 1], FP)
            nc.scalar.activation(junk, prod[:, 0:NB], AF.Identity,
                                 bias=0.0, scale=1.0, accum_out=mu16)
            junk2 = const.tile([BC, NB], FP)
            sgraw = const.tile([BC, 1], FP)
            nc.scalar.activation(junk2, prod[:, NB:2 * NB], AF.Identity,
                                 bias=0.0, scale=1.0, accum_out=sgraw)
            sigma16 = const.tile([BC, 1], FP)
            nc.vector.tensor_scalar(
                sigma16, sgraw, 0.0, MIN_SIGMA, op0=ALU.max, op1=ALU.add)
            nc.sync.dma_start(out=mu_o[:], in_=mu16)
            nc.sync.dma_start(out=sg_o[:], in_=sigma16)

            # conf = sigmoid(sigma * (-1/T) + relu(cb)/T)
            conf16 = const.tile([BC, 1], FP)
            nc.scalar.activation(
                conf16, sigma16, AF.Sigmoid,
                bias=scl_a[:, 1:2], scale=scl_a[:, 0:1])
            nc.sync.dma_start(out=cf_o[:], in_=conf16)

            # ---- per-partition params ----
            invsig = const.tile([BC, 1], FP)
            nc.vector.reciprocal(invsig, sigma16)
            b16 = const.tile([BC, 1], FP)
            nc.gpsimd.iota(b16, pattern=[[0, 1]], base=0, channel_multiplier=L,
                           allow_small_or_imprecise_dtypes=True)  # b*16384
            v16 = const.tile([BC, 4], FP)
            nc.vector.tensor_copy(v16[:, 0:1], lenf)
            nc.vector.tensor_mul(v16[:, 1:2], invl, invsig)   # s = 1/(len*sig)
            nc.vector.tensor_mul(v16[:, 2:3], mu16, invsig)   # mu/sig
            nc.vector.tensor_copy(v16[:, 3:4], b16)
            pp_ps = psp.tile([128, 4], FP)
            nc.tensor.matmul(pp_ps, lhsT=e8tc, rhs=v16, start=True, stop=True)
            pp = const.tile([128, 4], FP)
            nc.vector.tensor_copy(pp, pp_ps)

            # off_p = (p%8)*2048 = p*2048 - b(p)*16384
            p2048 = const.tile([128, 1], FP)
            nc.gpsimd.iota(p2048, pattern=[[0, 1]], base=0, channel_multiplier=CF,
                           allow_small_or_imprecise_dtypes=True)
            off = const.tile([128, 1], FP)
            nc.vector.tensor_sub(off, p2048, pp[:, 3:4])

            thrneg = const.tile([128, 1], FP)
            nc.vector.tensor_sub(thrneg, off, pp[:, 0:1])     # off - len
            biasp = const.tile([128, 1], FP)
            nc.vector.tensor_mul(biasp, off, pp[:, 1:2])
            nc.vector.tensor_sub(biasp, biasp, pp[:, 2:3])    # off*s - mu/sig
            sneg = const.tile([128, 1], FP)
            nc.vector.tensor_scalar(sneg, pp[:, 1:2], -1.0, None, op0=ALU.mult)

            # ---- big pass over [128, 2048], chunked to pipeline engines ----
            # fneg[.,f] = -(f+1); valid i < len  <=>  fneg >= off - len
            fneg = big.tile([128, CF], FP)
            nc.gpsimd.iota(fneg, pattern=[[-1, CF]], base=-1, channel_multiplier=0,
                           allow_small_or_imprecise_dtypes=True)
            scr_a = const.tile([1, 1], FP)
            nc.scalar.activation(  # pre-wait on Pool (fneg)
                scr_a, fneg[0:1, 0:1], AF.Identity, bias=0.0, scale=1.0)

            m_t = big.tile([128, CF], FP)
            z_t = big.tile([128, CF], FP)
            zm = big.tile([128, CF], FP)
            u2 = None if USE_DERF else big.tile([128, CF], FP)
            ps4 = const.tile([128, NCHUNK], FP)
            junk_c = [big.tile([128, CW], FP, name=f"junkc{c}")
                      for c in range(NCHUNK)]
            scr_ms = [const.tile([1, 1], FP, name=f"scr_m{c}")
                      for c in range(NCHUNK)]
            for c in range(NCHUNK):
                s = slice(c * CW, (c + 1) * CW)
                nc.gpsimd.tensor_scalar(
                    m_t[:, s], fneg[:, s], thrneg, None, op0=ALU.is_ge)
                if USE_DERF:
                    # DErf(x) = 2/sqrt(pi) * exp(-x^2); the constant cancels
                    # in the L1 normalization below.
                    nc.scalar.activation(
                        z_t[:, s], fneg[:, s], AF.Derivative_Erf,
                        bias=biasp, scale=sneg)
                else:
                    nc.scalar.activation(
                        u2[:, s], fneg[:, s], AF.Square, bias=biasp, scale=sneg)
                    nc.scalar.activation(z_t[:, s], u2[:, s], AF.Exp, scale=-1.0)
                nc.vector.tensor_copy(
                    scr_ms[c], m_t[0:1, s][:, 0:1])  # pre-wait Pool
                nc.vector.tensor_mul(zm[:, s], z_t[:, s], m_t[:, s])
                nc.scalar.activation(junk_c[c], zm[:, s], AF.Identity,
                                     bias=0.0, scale=1.0,
                                     accum_out=ps4[:, c:c + 1])

            s_ps = psp.tile([BC, NCHUNK], FP)
            nc.tensor.matmul(s_ps, lhsT=e8c, rhs=ps4, start=True, stop=True)
            junk4 = const.tile([BC, NCHUNK], FP)
            tot16 = const.tile([BC, 1], FP)
            nc.scalar.activation(junk4, s_ps, AF.Identity,
                                 bias=0.0, scale=1.0, accum_out=tot16)
            s16 = const.tile([BC, 1], FP)
            nc.vector.tensor_scalar(s16, tot16, 1e-12, None, op0=ALU.max)
            inv16 = const.tile([BC, 1], FP)
            nc.vector.reciprocal(inv16, s16)
            invp_ps = psp.tile([128, 1], FP)
            nc.tensor.matmul(invp_ps, lhsT=e8tc, rhs=inv16, start=True, stop=True)
            invp = const.tile([128, 1], FP)
            nc.vector.tensor_copy(invp, invp_ps)

            zn = big.tile([128, CF], FP)
            attn_ap = attn[:].rearrange("b (j f) -> (b j) f", j=J)
            for c in range(NCHUNK):
                s = slice(c * CW, (c + 1) * CW)
                nc.vector.tensor_scalar(
                    zn[:, s], zm[:, s], invp, None, op0=ALU.mult)
                nc.sync.dma_start(out=attn_ap[:, s], in_=zn[:, s])

    return nc


def _patch_bir(data):
    # This walrus build encodes at most one sync wait per instruction and
    # rejects the sequencer-only EVENT_SEMAPHORE_RANGE_CLEAR InstISA, both
    # emitted by the Tile teardown.  Split multi-wait drains into a chain
    # of single-wait drains and drop the range-clear (semaphores are reset
    # by the runtime between executions).
    for f in data["functions"]:
        for b in f["blocks"]:
            new_ins = []
            for ins in b["instructions"]:
                if (ins.get("opcode") == "ISA"
                        and ins.get("op_name") == "EVENT_SEMAPHORE_RANGE_CLEAR"):
                    continue
                si = ins.get("sync_info") or {}
                waits = si.get("on_wait") or []
                if len(waits) > 1:
                    for k, w in enumerate(waits[:-1]):
                        new_ins.append({
                            "debug": ins.get("debug", 0),
                            "engine": ins["engine"],
                            "ins": [], "outs": [],
                            "name": f"{ins['name']}w{k}",
                            "opcode": "Drain",
                            "sync_info": {"on_update": [], "on_wait": [w]},
                        })
                    ins = dict(ins)
                    ins["sync_info"] = {
                        "on_update": si.get("on_update") or [],
                        "on_wait": [waits[-1]],
                    }
                new_ins.append(ins)
            b["instructions"] = new_ins
    return data


_NC_CACHE = {}


def _get_nc():
    if "nc" not in _NC_CACHE:
        nc = _build_nc()
        orig = nc.to_json_bytes
        nc.to_json_bytes = lambda: orjson.dumps(
            _patch_bir(orjson.loads(orig())))
        _NC_CACHE["nc"] = nc
    return _NC_CACHE["nc"]


def make_in_maps(inputs):
    dec = np.asarray(inputs["decoder_outputs"], np.float32)      # (B,1,D)
    mu_old = np.asarray(inputs["mu_old"], np.float32).reshape(B)
    sg_old = np.asarray(inputs["sigma_old"], np.float32).reshape(B)
    mc_old = np.asarray(inputs["mean_content_old"], np.float32).reshape(B)
    ma_old = np.asarray(inputs["mean_attn_old"], np.float32).reshape(B)
    lens = np.asarray(inputs["source_lengths"], np.int32).reshape(B)
    step = int(np.asarray(inputs["step"]))
    conf_temp = float(np.asarray(inputs["conf_temp"]))
    conf_bias = float(np.asarray(inputs["conf_bias"]))

    w1cat = np.ascontiguousarray(
        np.concatenate([np.asarray(inputs["Wmu1"], np.float32),
                        np.asarray(inputs["Wsg1"], np.float32)], axis=1))
    w2blk = np.zeros((2 * H, 2 * NB), np.float32)
    w2blk[0:H, 0:NB] = np.asarray(inputs["Wmu2"], np.float32)
    w2blk[H:2 * H, NB:2 * NB] = np.asarray(inputs["Wsg2"], np.float32)
    b1cat = np.concatenate([np.asarray(inputs["bmu1"], np.float32),
                            np.asarray(inputs["bsg1"], np.float32)]).reshape(2 * H, 1)
    b2cat = np.concatenate([np.asarray(inputs["bmu2"], np.float32),
                            np.asarray(inputs["bsg2"], np.float32)]).reshape(2 * NB, 1)

    in_maps = []
    for c in range(NCORES):
        r = slice(c * BC, (c + 1) * BC)
        sclc = np.zeros((BC, 8), np.float32)
        sclc[:, 0] = mu_old[r]
        sclc[:, 1] = sg_old[r]
        sclc[:, 2] = mc_old[r]
        sclc[:, 3] = ma_old[r]
        sclc[:, 4] = lens[r].astype(np.float32)
        sclc[:, 5] = step + 1.0
        sclc[:, 6] = -1.0 / conf_temp
        sclc[:, 7] = max(conf_bias, 0.0) / conf_temp
        in_maps.append({
            "decT": np.ascontiguousarray(dec[r, 0, :].T),
            "w1": w1cat, "b1": b1cat, "w2": w2blk, "b2": b2cat,
            "scl": sclc,
        })
    return in_maps


def kernel(**inputs):
    in_maps = make_in_maps(inputs)
    res = run_bass_kernel_spmd(_get_nc(), in_maps, list(range(NCORES)))
    results = res.results

    pos_attn = np.concatenate([results[c]["attn"] for c in range(NCORES)],
                              axis=0).reshape(B, 1, L)
    mu = np.concatenate([results[c]["mu_o"] for c in range(NCORES)],
                        axis=0).reshape(B, 1, 1)
    sigma = np.concatenate([results[c]["sg_o"] for c in range(NCORES)],
                           axis=0).reshape(B, 1, 1)
    conf = np.concatenate([results[c]["cf_o"] for c in range(NCORES)],
                          axis=0).reshape(B, 1, 1)
    return pos_attn, conf, mu, sigma


# revision 41
# speedup vs baseline: 1.7584x; 1.7584x over previous
"""PositionAttention kernel for TRN2 (8 cores, data-parallel over batch).

Math (per batch row b):
  rel_dec  = (step+1)/len_b
  shared   = [mu_old, sigma_old/len, mc_old, ma_old, rel_dec]           (5)
  pos_in   = [dec(512), shared]                                        (517)
  blocks   = [shared, 1/len, -1/len, 1]                                  (8)
  mu_w     = relu(pos_in @ Wmu1 + bmu1) @ Wmu2 + bmu2                    (8)
  sg_w     = relu(pos_in @ Wsg1 + bsg1) @ Wsg2 + bsg2                    (8)
  mu       = sum(mu_w * blocks);  sigma = relu(sum(sg_w * blocks)) + .05
  conf     = sigmoid((-sigma + relu(conf_bias)) / conf_temp)
  z_i      = exp(-(((i+1)/len - mu)/sigma)^2) for i < len else 0
  attn     = z / max(sum(z), 1e-12)

Layout: 16 batch rows/core.  Big tile [128, 2048]: partition p = 8*b + j
(b local row, j L-chunk of 2048), free f -> position i = 2048*j + f.
"""

import numpy as np
import orjson

import concourse.bass as bass
import concourse.tile as tile
from concourse import mybir
from concourse.bass_utils import run_bass_kernel_spmd

B, L, D, H, NB = 128, 16384, 512, 32, 8
NCORES = 8
BC = B // NCORES          # 16 batch rows per core
J = 8                     # L-chunks per row
CF = L // J               # 2048 free elems per chunk
P = BC * J                # 128 partitions
IN = D + 5                # 517
MIN_SIGMA = 0.05

FP = mybir.dt.float32
AF = mybir.ActivationFunctionType
ALU = mybir.AluOpType

NCHUNK = 4
CW = CF // NCHUNK         # 512 free elems per chunk
USE_DERF = True           # fused exp(-x^2) via Derivative_Erf (not in sim)


def _build_nc():
    nc = bass.Bass("TRN2")

    decT = nc.dram_tensor("decT", [D, BC], FP, kind="ExternalInput")
    w1 = nc.dram_tensor("w1", [IN, 2 * H], FP, kind="ExternalInput")
    b1 = nc.dram_tensor("b1", [2 * H, 1], FP, kind="ExternalInput")
    w2 = nc.dram_tensor("w2", [2 * H, 2 * NB], FP, kind="ExternalInput")
    b2 = nc.dram_tensor("b2", [2 * NB, 1], FP, kind="ExternalInput")
    # scl[b] = [mu_old, sg_old, mc_old, ma_old, len,
    #           step+1, -1/T, relu(cb)/T]
    scl = nc.dram_tensor("scl", [BC, 8], FP, kind="ExternalInput")
    # precomputed constants: sm16 = [id16 | b*L | e8t], sm128 = [e8 | p*CF],
    # cst = fneg ramp -(f+1) broadcast over partitions
    sm16d = nc.dram_tensor("sm16", [BC, 145], FP, kind="ExternalInput")
    sm128d = nc.dram_tensor("sm128", [128, 17], FP, kind="ExternalInput")
    cstd = nc.dram_tensor("cst", [128, CF], FP, kind="ExternalInput")

    attn = nc.dram_tensor("attn", [BC, L], FP, kind="ExternalOutput")
    mu_o = nc.dram_tensor("mu_o", [BC, 1], FP, kind="ExternalOutput")
    sg_o = nc.dram_tensor("sg_o", [BC, 1], FP, kind="ExternalOutput")
    cf_o = nc.dram_tensor("cf_o", [BC, 1], FP, kind="ExternalOutput")

    with tile.TileContext(nc) as tc:
        with (
            tc.tile_pool(name="const", bufs=1) as const,
            tc.tile_pool(name="big", bufs=1) as big,
            tc.psum_pool(name="ps", bufs=1) as psp,
        ):
            # ---- load small inputs (coalesced DMAs) ----
            sb_decT = const.tile([128, 4 * BC], FP)
            nc.sync.dma_start(
                out=sb_decT.rearrange("p (k n) -> p k n", k=4),
                in_=decT[:].rearrange("(k p) n -> p k n", k=4),
            )
            sb_w1 = const.tile([128, 4 * 2 * H], FP)
            nc.sync.dma_start(
                out=sb_w1.rearrange("p (k h) -> p k h", k=4),
                in_=w1[0:512, :].rearrange("(k p) h -> p k h", k=4),
            )
            sb_w1t = const.tile([5, 2 * H], FP)
            nc.sync.dma_start(out=sb_w1t, in_=w1[4 * 128:IN, :])
            sb_w2 = const.tile([2 * H, 2 * NB], FP)
            nc.sync.dma_start(out=sb_w2, in_=w2[:])
            sb_b1 = const.tile([2 * H, 1], FP)
            nc.sync.dma_start(out=sb_b1, in_=b1[:])
            sb_b2 = const.tile([2 * NB, 1], FP)
            nc.sync.dma_start(out=sb_b2, in_=b2[:])
            sb_scl = const.tile([BC, 8], FP)
            nc.sync.dma_start(out=sb_scl, in_=scl[:])
            sb_sm16 = const.tile([BC, 145], FP)
            nc.sync.dma_start(out=sb_sm16, in_=sm16d[:])
            sb_sm128 = const.tile([128, 17], FP)
            nc.sync.dma_start(out=sb_sm128, in_=sm128d[:])

            # Stage all PE inputs through DVE copies: a Matmult's codegen
            # (fp32 LW expansion) overflows its sync-wait slots when waiting
            # directly on DMA-queue semaphores, so PE must only ever wait on
            # engine semaphores.
            decTc = const.tile([128, 4 * BC], FP)
            nc.vector.tensor_copy(decTc, sb_decT)
            w1c = const.tile([128, 4 * 2 * H], FP)
            nc.vector.tensor_copy(w1c, sb_w1)
            w1tc = const.tile([5, 2 * H], FP)
            nc.vector.tensor_copy(w1tc, sb_w1t)
            w2c = const.tile([2 * H, 2 * NB], FP)
            nc.vector.tensor_copy(w2c, sb_w2)

            # ---- per-b scalars, column layout [16,1] ----
            lenf = sb_scl[:, 4:5]
            invl = const.tile([BC, 1], FP)
            nc.vector.reciprocal(invl, lenf)

            # blocks matrix, column layout [16(b), 8]:
            # [mu_old, sg_old/len, mc_old, ma_old, (step+1)/len, 1/len, -1/len, 1]
            bcol = const.tile([BC, NB], FP)
            nc.vector.tensor_copy(bcol[:, 0:4], sb_scl[:, 0:4])
            nc.vector.tensor_mul(bcol[:, 1:2], sb_scl[:, 1:2], invl)
            nc.vector.tensor_scalar(
                bcol[:, 4:5], invl, sb_scl[:, 5:6], None, op0=ALU.mult)
            nc.vector.tensor_copy(bcol[:, 5:6], invl)
            nc.vector.tensor_scalar(
                bcol[:, 6:7], invl, -1.0, None, op0=ALU.mult)
            nc.vector.memset(bcol[:, 7:8], 1.0)

            # ---- constant matrices (DMA'd; DVE-staged for PE's 1-wait rule) ----
            id16c = const.tile([BC, BC], FP)
            nc.vector.tensor_copy(id16c, sb_sm16[:, 0:BC])
            e8tc = const.tile([BC, 128], FP)
            nc.vector.tensor_copy(e8tc, sb_sm16[:, 17:145])
            e8c = const.tile([128, BC], FP)
            nc.vector.tensor_copy(e8c, sb_sm128[:, 0:BC])

            # ACT staging: every compute instruction may carry at most ONE
            # sync wait, so ACT pre-stages its bias operands (one DMA wait
            # each) and later ACT ops wait only on their tensor producer.
            b1a = const.tile([2 * H, 1], FP)
            nc.scalar.activation(b1a, sb_b1, AF.Identity, bias=0.0, scale=1.0)
            b2a = const.tile([2 * NB, 1], FP)
            nc.scalar.activation(b2a, sb_b2, AF.Identity, bias=0.0, scale=1.0)
            scl_a = const.tile([BC, 2], FP)
            nc.scalar.activation(
                scl_a, sb_scl[:, 6:8], AF.Identity, bias=0.0, scale=1.0)
            # pre-wait: one ACT op waiting on the last ACT staging op, so
            # later ACT ops' own-engine bias deps are elided (max-count rule)
            scr_s = const.tile([1, 1], FP)
            nc.scalar.activation(
                scr_s, scl_a[0:1, 0:1], AF.Identity, bias=0.0, scale=1.0)

            # shared.T [5,16] row layout for the MLP tail matmul
            shr_ps = psp.tile([5, BC], FP)
            nc.tensor.transpose(shr_ps, bcol[:, 0:5], id16c)
            shr_sb = const.tile([5, BC], FP)
            nc.vector.tensor_copy(shr_sb, shr_ps)

            # ---- MLP: H1 = W1cat.T @ pos_inT  -> [64, 16] ----
            h1_ps = psp.tile([2 * H, BC], FP)
            for k in range(4):
                nc.tensor.matmul(
                    h1_ps,
                    lhsT=w1c[:, k * 2 * H:(k + 1) * 2 * H],
                    rhs=decTc[:, k * BC:(k + 1) * BC],
                    start=(k == 0), stop=False)
            nc.tensor.matmul(
                h1_ps, lhsT=w1tc, rhs=shr_sb, start=False, stop=True)
            a_sb = const.tile([2 * H, BC], FP)
            nc.scalar.activation(a_sb, h1_ps, AF.Relu, bias=b1a, scale=1.0)

            # WS = W2blk.T @ A -> [16,16] (rows: 8 mu weights, 8 sg weights)
            ws_ps = psp.tile([2 * NB, BC], FP)
            nc.tensor.matmul(ws_ps, lhsT=w2c, rhs=a_sb, start=True, stop=True)
            wsb = const.tile([2 * NB, BC], FP)
            nc.scalar.activation(wsb, ws_ps, AF.Identity, bias=b2a, scale=1.0)

            # transpose -> [16(b), 16(weights)]
            wt_ps = psp.tile([BC, BC], FP)
            nc.tensor.transpose(wt_ps, wsb, id16c)

            # TensorTensorReduce serializes as a raw-ISA instruction this
            # walrus build rejects; emulate with TensorTensor mult + an ACT
            # Identity pass whose accum_out does the free-dim reduction.
            bcol2 = const.tile([BC, 2 * NB], FP)
            nc.vector.tensor_copy(bcol2[:, 0:NB], bcol)
            nc.vector.tensor_copy(bcol2[:, NB:2 * NB], bcol)
            scr_d = const.tile([1, 1], FP)
            nc.vector.tensor_copy(scr_d, wt_ps[0:1, 0:1])  # pre-wait on PE
            prod = const.tile([BC, 2 * NB], FP)
            nc.vector.tensor_mul(prod, wt_ps, bcol2)
            junk = const.tile([BC, NB], FP)
            mu16 = const.tile([BC, 1], FP)
            nc.scalar.activation(junk, prod[:, 0:NB], AF.Identity,
                                 bias=0.0, scale=1.0, accum_out=mu16)
            junk2 = const.tile([BC, NB], FP)
            sgraw = const.tile([BC, 1], FP)
            nc.scalar.activation(junk2, prod[:, NB:2 * NB], AF.Identity,
                                 bias=0.0, scale=1.0, accum_out=sgraw)
            sigma16 = const.tile([BC, 1], FP)
            nc.vector.tensor_scalar(
                sigma16, sgraw, 0.0, MIN_SIGMA, op0=ALU.max, op1=ALU.add)
            nc.sync.dma_start(out=mu_o[:], in_=mu16)
            nc.sync.dma_start(out=sg_o[:], in_=sigma16)

            # conf = sigmoid(sigma * (-1/T) + relu(cb)/T)
            conf16 = const.tile([BC, 1], FP)
            nc.scalar.activation(
                conf16, sigma16, AF.Sigmoid,
                bias=scl_a[:, 1:2], scale=scl_a[:, 0:1])
            nc.sync.dma_start(out=cf_o[:], in_=conf16)

            # ---- per-partition params ----
            invsig = const.tile([BC, 1], FP)
            nc.vector.reciprocal(invsig, sigma16)
            v16 = const.tile([BC, 4], FP)
            nc.vector.tensor_copy(v16[:, 0:1], lenf)
            nc.vector.tensor_mul(v16[:, 1:2], invl, invsig)   # s = 1/(len*sig)
            nc.vector.tensor_mul(v16[:, 2:3], mu16, invsig)   # mu/sig
            nc.vector.tensor_copy(v16[:, 3:4], sb_sm16[:, 16:17])  # b*16384
            pp_ps = psp.tile([128, 4], FP)
            nc.tensor.matmul(pp_ps, lhsT=e8tc, rhs=v16, start=True, stop=True)
            pp = const.tile([128, 4], FP)
            nc.vector.tensor_copy(pp, pp_ps)

            # off_p = (p%8)*2048 = p*2048 - b(p)*16384
            off = const.tile([128, 1], FP)
            nc.vector.tensor_sub(off, sb_sm128[:, 16:17], pp[:, 3:4])

            thrneg = const.tile([128, 1], FP)
            nc.vector.tensor_sub(thrneg, off, pp[:, 0:1])     # off - len
            biasp = const.tile([128, 1], FP)
            nc.vector.tensor_mul(biasp, off, pp[:, 1:2])
            nc.vector.tensor_sub(biasp, biasp, pp[:, 2:3])    # off*s - mu/sig
            sneg = const.tile([128, 1], FP)
            nc.vector.tensor_scalar(sneg, pp[:, 1:2], -1.0, None, op0=ALU.mult)

            # ---- big pass over [128, 2048], chunked to pipeline engines ----
            # fneg[.,f] = -(f+1); valid i < len  <=>  fneg >= off - len
            fneg = big.tile([128, CF], FP)
            nc.sync.dma_start(out=fneg, in_=cstd[:])
            scr_a = const.tile([1, 1], FP)
            nc.scalar.activation(  # pre-wait on fneg's DMA queue
                scr_a, fneg[0:1, 0:1], AF.Identity, bias=0.0, scale=1.0)

            m_t = big.tile([128, CF], FP)
            z_t = big.tile([128, CF], FP)
            zm = big.tile([128, CF], FP)
            u2 = None if USE_DERF else big.tile([128, CF], FP)
            ps4 = const.tile([128, NCHUNK], FP)
            junk_c = [big.tile([128, CW], FP, name=f"junkc{c}")
                      for c in range(NCHUNK)]
            for c in range(NCHUNK):
                s = slice(c * CW, (c + 1) * CW)
                nc.vector.tensor_scalar(
                    m_t[:, s], fneg[:, s], thrneg, None, op0=ALU.is_ge)
                if USE_DERF:
                    # DErf(x) = 2/sqrt(pi) * exp(-x^2); the constant cancels
                    # in the L1 normalization below.
                    nc.scalar.activation(
                        z_t[:, s], fneg[:, s], AF.Derivative_Erf,
                        bias=biasp, scale=sneg)
                else:
                    nc.scalar.activation(
                        u2[:, s], fneg[:, s], AF.Square, bias=biasp, scale=sneg)
                    nc.scalar.activation(z_t[:, s], u2[:, s], AF.Exp, scale=-1.0)
                nc.vector.tensor_mul(zm[:, s], z_t[:, s], m_t[:, s])
                nc.scalar.activation(junk_c[c], zm[:, s], AF.Identity,
                                     bias=0.0, scale=1.0,
                                     accum_out=ps4[:, c:c + 1])

            s_ps = psp.tile([BC, NCHUNK], FP)
            nc.tensor.matmul(s_ps, lhsT=e8c, rhs=ps4, start=True, stop=True)
            junk4 = const.tile([BC, NCHUNK], FP)
            tot16 = const.tile([BC, 1], FP)
            nc.scalar.activation(junk4, s_ps, AF.Identity,
                                 bias=0.0, scale=1.0, accum_out=tot16)
            s16 = const.tile([BC, 1], FP)
            nc.vector.tensor_scalar(s16, tot16, 1e-12, None, op0=ALU.max)
            inv16 = const.tile([BC, 1], FP)
            nc.vector.reciprocal(inv16, s16)
            invp_ps = psp.tile([128, 1], FP)
            nc.tensor.matmul(invp_ps, lhsT=e8tc, rhs=inv16, start=True, stop=True)
            invp = const.tile([128, 1], FP)
            nc.vector.tensor_copy(invp, invp_ps)

            zn = big.tile([128, CF], FP)
            attn_ap = attn[:].rearrange("b (j f) -> (b j) f", j=J)
            for c in range(NCHUNK):
                s = slice(c * CW, (c + 1) * CW)
                nc.vector.tensor_scalar(
                    zn[:, s], zm[:, s], invp, None, op0=ALU.mult)
                nc.sync.dma_start(out=attn_ap[:, s], in_=zn[:, s])

    return nc


def _patch_bir(data):
    # This walrus build encodes at most one sync wait per instruction and
    # rejects the sequencer-only EVENT_SEMAPHORE_RANGE_CLEAR InstISA, both
    # emitted by the Tile teardown.  Split multi-wait drains into a chain
    # of single-wait drains and drop the range-clear (semaphores are reset
    # by the runtime between executions).
    for f in data["functions"]:
        for b in f["blocks"]:
            new_ins = []
            for ins in b["instructions"]:
                if (ins.get("opcode") == "ISA"
                        and ins.get("op_name") == "EVENT_SEMAPHORE_RANGE_CLEAR"):
                    continue
                si = ins.get("sync_info") or {}
                waits = si.get("on_wait") or []
                if len(waits) > 1:
                    for k, w in enumerate(waits[:-1]):
                        new_ins.append({
                            "debug": ins.get("debug", 0),
                            "engine": ins["engine"],
                            "ins": [], "outs": [],
                            "name": f"{ins['name']}w{k}",
                            "opcode": "Drain",
                            "sync_info": {"on_update": [], "on_wait": [w]},
                        })
                    ins = dict(ins)
                    ins["sync_info"] = {
                        "on_update": si.get("on_update") or [],
                        "on_wait": [waits[-1]],
                    }
                new_ins.append(ins)
            b["instructions"] = new_ins
    return data


_NC_CACHE = {}


def _get_nc():
    if "nc" not in _NC_CACHE:
        nc = _build_nc()
        orig = nc.to_json_bytes
        nc.to_json_bytes = lambda: orjson.dumps(
            _patch_bir(orjson.loads(orig())))
        _NC_CACHE["nc"] = nc
    return _NC_CACHE["nc"]


def make_in_maps(inputs):
    dec = np.asarray(inputs["decoder_outputs"], np.float32)      # (B,1,D)
    mu_old = np.asarray(inputs["mu_old"], np.float32).reshape(B)
    sg_old = np.asarray(inputs["sigma_old"], np.float32).reshape(B)
    mc_old = np.asarray(inputs["mean_content_old"], np.float32).reshape(B)
    ma_old = np.asarray(inputs["mean_attn_old"], np.float32).reshape(B)
    lens = np.asarray(inputs["source_lengths"], np.int32).reshape(B)
    step = int(np.asarray(inputs["step"]))
    conf_temp = float(np.asarray(inputs["conf_temp"]))
    conf_bias = float(np.asarray(inputs["conf_bias"]))

    w1cat = np.ascontiguousarray(
        np.concatenate([np.asarray(inputs["Wmu1"], np.float32),
                        np.asarray(inputs["Wsg1"], np.float32)], axis=1))
    w2blk = np.zeros((2 * H, 2 * NB), np.float32)
    w2blk[0:H, 0:NB] = np.asarray(inputs["Wmu2"], np.float32)
    w2blk[H:2 * H, NB:2 * NB] = np.asarray(inputs["Wsg2"], np.float32)
    b1cat = np.concatenate([np.asarray(inputs["bmu1"], np.float32),
                            np.asarray(inputs["bsg1"], np.float32)]).reshape(2 * H, 1)
    b2cat = np.concatenate([np.asarray(inputs["bmu2"], np.float32),
                            np.asarray(inputs["bsg2"], np.float32)]).reshape(2 * NB, 1)

    id16 = np.eye(BC, dtype=np.float32)
    b16 = (np.arange(BC, dtype=np.float32) * L).reshape(BC, 1)
    e8t = np.zeros((BC, 128), np.float32)
    for b_ in range(BC):
        e8t[b_, b_ * J:(b_ + 1) * J] = 1.0
    sm16 = np.ascontiguousarray(
        np.concatenate([id16, b16, e8t], axis=1))            # [16,145]
    p2048 = (np.arange(128, dtype=np.float32) * CF).reshape(128, 1)
    sm128 = np.ascontiguousarray(
        np.concatenate([e8t.T, p2048], axis=1))              # [128,17]
    fneg = np.ascontiguousarray(
        -np.tile(np.arange(1, CF + 1, dtype=np.float32), (128, 1)))

    in_maps = []
    for c in range(NCORES):
        r = slice(c * BC, (c + 1) * BC)
        sclc = np.zeros((BC, 8), np.float32)
        sclc[:, 0] = mu_old[r]
        sclc[:, 1] = sg_old[r]
        sclc[:, 2] = mc_old[r]
        sclc[:, 3] = ma_old[r]
        sclc[:, 4] = lens[r].astype(np.float32)
        sclc[:, 5] = step + 1.0
        sclc[:, 6] = -1.0 / conf_temp
        sclc[:, 7] = max(conf_bias, 0.0) / conf_temp
        in_maps.append({
            "decT": np.ascontiguousarray(dec[r, 0, :].T),
            "w1": w1cat, "b1": b1cat, "w2": w2blk, "b2": b2cat,
            "scl": sclc, "sm16": sm16, "sm128": sm128, "cst": fneg,
        })
    return in_maps


def kernel(**inputs):
    in_maps = make_in_maps(inputs)
    res = run_bass_kernel_spmd(_get_nc(), in_maps, list(range(NCORES)))
    results = res.results

    pos_attn = np.concatenate([results[c]["attn"] for c in range(NCORES)],
                              axis=0).reshape(B, 1, L)
    mu = np.concatenate([results[c]["mu_o"] for c in range(NCORES)],
                        axis=0).reshape(B, 1, 1)
    sigma = np.concatenate([results[c]["sg_o"] for c in range(NCORES)],
                           axis=0).reshape(B, 1, 1)
    conf = np.concatenate([results[c]["cf_o"] for c in range(NCORES)],
                          axis=0).reshape(B, 1, 1)
    return pos_attn, conf, mu, sigma


# revision 43
# speedup vs baseline: 1.8974x; 1.0791x over previous
"""PositionAttention kernel for TRN2 (8 cores, data-parallel over batch).

Math (per batch row b):
  rel_dec  = (step+1)/len_b
  shared   = [mu_old, sigma_old/len, mc_old, ma_old, rel_dec]           (5)
  pos_in   = [dec(512), shared]                                        (517)
  blocks   = [shared, 1/len, -1/len, 1]                                  (8)
  mu_w     = relu(pos_in @ Wmu1 + bmu1) @ Wmu2 + bmu2                    (8)
  sg_w     = relu(pos_in @ Wsg1 + bsg1) @ Wsg2 + bsg2                    (8)
  mu       = sum(mu_w * blocks);  sigma = relu(sum(sg_w * blocks)) + .05
  conf     = sigmoid((-sigma + relu(conf_bias)) / conf_temp)
  z_i      = exp(-(((i+1)/len - mu)/sigma)^2) for i < len else 0
  attn     = z / max(sum(z), 1e-12)

Layout: 16 batch rows/core.  Big tile [128, 2048]: partition p = 8*b + j
(b local row, j L-chunk of 2048), free f -> position i = 2048*j + f.
"""

import numpy as np
import orjson

import concourse.bass as bass
import concourse.tile as tile
from concourse import mybir
from concourse.bass_utils import run_bass_kernel_spmd

B, L, D, H, NB = 128, 16384, 512, 32, 8
NCORES = 8
BC = B // NCORES          # 16 batch rows per core
J = 8                     # L-chunks per row
CF = L // J               # 2048 free elems per chunk
P = BC * J                # 128 partitions
IN = D + 5                # 517
MIN_SIGMA = 0.05

FP = mybir.dt.float32
AF = mybir.ActivationFunctionType
ALU = mybir.AluOpType

NCHUNK = 4
CW = CF // NCHUNK         # 512 free elems per chunk
USE_DERF = True           # fused exp(-x^2) via Derivative_Erf (not in sim)


def _build_nc():
    nc = bass.Bass("TRN2")

    decT = nc.dram_tensor("decT", [D, BC], FP, kind="ExternalInput")
    w1 = nc.dram_tensor("w1", [IN, 2 * H], FP, kind="ExternalInput")
    b1 = nc.dram_tensor("b1", [2 * H, 1], FP, kind="ExternalInput")
    w2 = nc.dram_tensor("w2", [2 * H, 2 * NB], FP, kind="ExternalInput")
    b2 = nc.dram_tensor("b2", [2 * NB, 1], FP, kind="ExternalInput")
    # scl[b] = [mu_old, sg_old, mc_old, ma_old, len,
    #           step+1, -1/T, relu(cb)/T]
    scl = nc.dram_tensor("scl", [BC, 8], FP, kind="ExternalInput")
    # precomputed constants: sm16 = [id16 | b*L | e8t], sm128 = [e8 | p*CF],
    # cst = fneg ramp -(f+1) broadcast over partitions
    sm16d = nc.dram_tensor("sm16", [BC, 145], FP, kind="ExternalInput")
    sm128d = nc.dram_tensor("sm128", [128, 17], FP, kind="ExternalInput")
    cstd = nc.dram_tensor("cst", [128, CF], FP, kind="ExternalInput")

    attn = nc.dram_tensor("attn", [BC, L], FP, kind="ExternalOutput")
    mu_o = nc.dram_tensor("mu_o", [BC, 1], FP, kind="ExternalOutput")
    sg_o = nc.dram_tensor("sg_o", [BC, 1], FP, kind="ExternalOutput")
    cf_o = nc.dram_tensor("cf_o", [BC, 1], FP, kind="ExternalOutput")

    with tile.TileContext(nc) as tc:
        with (
            tc.tile_pool(name="const", bufs=1) as const,
            tc.tile_pool(name="big", bufs=1) as big,
            tc.psum_pool(name="ps", bufs=1) as psp,
        ):
            # ---- load small inputs (coalesced DMAs) ----
            sb_decT = const.tile([128, 4 * BC], FP)
            nc.sync.dma_start(
                out=sb_decT.rearrange("p (k n) -> p k n", k=4),
                in_=decT[:].rearrange("(k p) n -> p k n", k=4),
            )
            sb_w1 = const.tile([128, 4 * 2 * H], FP)
            nc.sync.dma_start(
                out=sb_w1.rearrange("p (k h) -> p k h", k=4),
                in_=w1[0:512, :].rearrange("(k p) h -> p k h", k=4),
            )
            sb_w1t = const.tile([5, 2 * H], FP)
            nc.sync.dma_start(out=sb_w1t, in_=w1[4 * 128:IN, :])
            sb_w2 = const.tile([2 * H, 2 * NB], FP)
            nc.sync.dma_start(out=sb_w2, in_=w2[:])
            sb_b1 = const.tile([2 * H, 1], FP)
            nc.sync.dma_start(out=sb_b1, in_=b1[:])
            sb_b2 = const.tile([2 * NB, 1], FP)
            nc.sync.dma_start(out=sb_b2, in_=b2[:])
            sb_scl = const.tile([BC, 8], FP)
            nc.sync.dma_start(out=sb_scl, in_=scl[:])
            sb_sm16 = const.tile([BC, 145], FP)
            nc.sync.dma_start(out=sb_sm16, in_=sm16d[:])
            sb_sm128 = const.tile([128, 17], FP)
            nc.sync.dma_start(out=sb_sm128, in_=sm128d[:])

            # Stage all PE inputs through DVE copies: a Matmult's codegen
            # (fp32 LW expansion) overflows its sync-wait slots when waiting
            # directly on DMA-queue semaphores, so PE must only ever wait on
            # engine semaphores.
            decTc = const.tile([128, 4 * BC], FP)
            nc.vector.tensor_copy(decTc, sb_decT)
            w1c = const.tile([128, 4 * 2 * H], FP)
            nc.vector.tensor_copy(w1c, sb_w1)
            w1tc = const.tile([5, 2 * H], FP)
            nc.vector.tensor_copy(w1tc, sb_w1t)
            w2c = const.tile([2 * H, 2 * NB], FP)
            nc.vector.tensor_copy(w2c, sb_w2)

            # ---- per-b scalars, column layout [16,1] ----
            lenf = sb_scl[:, 4:5]
            invl = const.tile([BC, 1], FP)
            nc.vector.reciprocal(invl, lenf)

            # blocks matrix, column layout [16(b), 8]:
            # [mu_old, sg_old/len, mc_old, ma_old, (step+1)/len, 1/len, -1/len, 1]
            bcol = const.tile([BC, NB], FP)
            nc.vector.tensor_copy(bcol[:, 0:4], sb_scl[:, 0:4])
            nc.vector.tensor_mul(bcol[:, 1:2], sb_scl[:, 1:2], invl)
            nc.vector.tensor_scalar(
                bcol[:, 4:5], invl, sb_scl[:, 5:6], None, op0=ALU.mult)
            nc.vector.tensor_copy(bcol[:, 5:6], invl)
            nc.vector.tensor_scalar(
                bcol[:, 6:7], invl, -1.0, None, op0=ALU.mult)
            nc.vector.memset(bcol[:, 7:8], 1.0)

            # ---- constant matrices (DMA'd; DVE-staged for PE's 1-wait rule) ----
            id16c = const.tile([BC, BC], FP)
            nc.vector.tensor_copy(id16c, sb_sm16[:, 0:BC])
            e8tc = const.tile([BC, 128], FP)
            nc.vector.tensor_copy(e8tc, sb_sm16[:, 17:145])
            e8c = const.tile([128, BC], FP)
            nc.vector.tensor_copy(e8c, sb_sm128[:, 0:BC])

            # ACT staging: every compute instruction may carry at most ONE
            # sync wait, so ACT pre-stages its bias operands (one DMA wait
            # each) and later ACT ops wait only on their tensor producer.
            b1a = const.tile([2 * H, 1], FP)
            nc.scalar.activation(b1a, sb_b1, AF.Identity, bias=0.0, scale=1.0)
            b2a = const.tile([2 * NB, 1], FP)
            nc.scalar.activation(b2a, sb_b2, AF.Identity, bias=0.0, scale=1.0)
            scl_a = const.tile([BC, 2], FP)
            nc.scalar.activation(
                scl_a, sb_scl[:, 6:8], AF.Identity, bias=0.0, scale=1.0)
            # pre-wait: one ACT op waiting on the last ACT staging op, so
            # later ACT ops' own-engine bias deps are elided (max-count rule)
            scr_s = const.tile([1, 1], FP)
            nc.scalar.activation(
                scr_s, scl_a[0:1, 0:1], AF.Identity, bias=0.0, scale=1.0)

            # shared.T [5,16] row layout for the MLP tail matmul
            shr_ps = psp.tile([5, BC], FP)
            nc.tensor.transpose(shr_ps, bcol[:, 0:5], id16c)
            shr_sb = const.tile([5, BC], FP)
            nc.vector.tensor_copy(shr_sb, shr_ps)

            # ---- MLP: H1 = W1cat.T @ pos_inT  -> [64, 16] ----
            h1_ps = psp.tile([2 * H, BC], FP)
            for k in range(4):
                nc.tensor.matmul(
                    h1_ps,
                    lhsT=w1c[:, k * 2 * H:(k + 1) * 2 * H],
                    rhs=decTc[:, k * BC:(k + 1) * BC],
                    start=(k == 0), stop=False)
            nc.tensor.matmul(
                h1_ps, lhsT=w1tc, rhs=shr_sb, start=False, stop=True)
            a_sb = const.tile([2 * H, BC], FP)
            nc.scalar.activation(a_sb, h1_ps, AF.Relu, bias=b1a, scale=1.0)

            # WS = W2blk.T @ A -> [16,16] (rows: 8 mu weights, 8 sg weights)
            ws_ps = psp.tile([2 * NB, BC], FP)
            nc.tensor.matmul(ws_ps, lhsT=w2c, rhs=a_sb, start=True, stop=True)
            wsb = const.tile([2 * NB, BC], FP)
            nc.scalar.activation(wsb, ws_ps, AF.Identity, bias=b2a, scale=1.0)

            # transpose -> [16(b), 16(weights)]
            wt_ps = psp.tile([BC, BC], FP)
            nc.tensor.transpose(wt_ps, wsb, id16c)

            # TensorTensorReduce serializes as a raw-ISA instruction this
            # walrus build rejects; emulate with TensorTensor mult + an ACT
            # Identity pass whose accum_out does the free-dim reduction.
            bcol2 = const.tile([BC, 2 * NB], FP)
            nc.vector.tensor_copy(bcol2[:, 0:NB], bcol)
            nc.vector.tensor_copy(bcol2[:, NB:2 * NB], bcol)
            scr_d = const.tile([1, 1], FP)
            nc.vector.tensor_copy(scr_d, wt_ps[0:1, 0:1])  # pre-wait on PE
            prod = const.tile([BC, 2 * NB], FP)
            nc.vector.tensor_mul(prod, wt_ps, bcol2)
            junk = const.tile([BC, NB], FP)
            mu16 = const.tile([BC, 1], FP)
            nc.scalar.activation(junk, prod[:, 0:NB], AF.Identity,
                                 bias=0.0, scale=1.0, accum_out=mu16)
            junk2 = const.tile([BC, NB], FP)
            sgraw = const.tile([BC, 1], FP)
            nc.scalar.activation(junk2, prod[:, NB:2 * NB], AF.Identity,
                                 bias=0.0, scale=1.0, accum_out=sgraw)
            sigma16 = const.tile([BC, 1], FP)
            nc.vector.tensor_scalar(
                sigma16, sgraw, 0.0, MIN_SIGMA, op0=ALU.max, op1=ALU.add)
            nc.sync.dma_start(out=mu_o[:], in_=mu16)
            nc.sync.dma_start(out=sg_o[:], in_=sigma16)

            # conf = sigmoid(sigma * (-1/T) + relu(cb)/T)
            conf16 = const.tile([BC, 1], FP)
            nc.scalar.activation(
                conf16, sigma16, AF.Sigmoid,
                bias=scl_a[:, 1:2], scale=scl_a[:, 0:1])
            nc.sync.dma_start(out=cf_o[:], in_=conf16)

            # ---- per-partition params ----
            invsig = const.tile([BC, 1], FP)
            nc.vector.reciprocal(invsig, sigma16)
            v16 = const.tile([BC, 4], FP)
            nc.vector.tensor_copy(v16[:, 0:1], lenf)
            nc.vector.tensor_mul(v16[:, 1:2], invl, invsig)   # s = 1/(len*sig)
            nc.vector.tensor_mul(v16[:, 2:3], mu16, invsig)   # mu/sig
            nc.vector.tensor_copy(v16[:, 3:4], sb_sm16[:, 16:17])  # b*16384
            pp_ps = psp.tile([128, 4], FP)
            nc.tensor.matmul(pp_ps, lhsT=e8tc, rhs=v16, start=True, stop=True)
            pp = const.tile([128, 4], FP)
            nc.vector.tensor_copy(pp, pp_ps)

            # off_p = (p%8)*2048 = p*2048 - b(p)*16384
            off = const.tile([128, 1], FP)
            nc.vector.tensor_sub(off, sb_sm128[:, 16:17], pp[:, 3:4])

            thrneg = const.tile([128, 1], FP)
            nc.vector.tensor_sub(thrneg, off, pp[:, 0:1])     # off - len
            biasp = const.tile([128, 1], FP)
            nc.vector.tensor_mul(biasp, off, pp[:, 1:2])
            nc.vector.tensor_sub(biasp, biasp, pp[:, 2:3])    # off*s - mu/sig
            sneg = const.tile([128, 1], FP)
            nc.vector.tensor_scalar(sneg, pp[:, 1:2], -1.0, None, op0=ALU.mult)

            # ---- big pass over [128, 2048], chunked to pipeline engines ----
            # fneg[.,f] = -(f+1); valid i < len  <=>  fneg >= off - len
            fneg = big.tile([128, CF], FP)
            nc.sync.dma_start(out=fneg, in_=cstd[:])
            scr_a = const.tile([1, 1], FP)
            nc.scalar.activation(  # pre-wait on fneg's DMA queue
                scr_a, fneg[0:1, 0:1], AF.Identity, bias=0.0, scale=1.0)

            m_t = big.tile([128, CF], FP)
            z_t = big.tile([128, CF], FP)
            zm = big.tile([128, CF], FP)
            u2 = None if USE_DERF else big.tile([128, CF], FP)
            ps4 = const.tile([128, NCHUNK], FP)
            for c in range(NCHUNK):
                s = slice(c * CW, (c + 1) * CW)
                # mask folded into the DErf input: invalid (i >= len)
                # positions get +1e9, so |x'| is huge and exp(-x'^2) = 0;
                # lets the DErf op itself carry the free-dim accumulation.
                nc.vector.tensor_scalar(
                    m_t[:, s], fneg[:, s], thrneg, 1e9,
                    op0=ALU.is_lt, op1=ALU.mult)
                nc.vector.tensor_add(zm[:, s], fneg[:, s], m_t[:, s])
                if USE_DERF:
                    # DErf(x) = 2/sqrt(pi) * exp(-x^2); the constant cancels
                    # in the L1 normalization below.
                    nc.scalar.activation(
                        z_t[:, s], zm[:, s], AF.Derivative_Erf,
                        bias=biasp, scale=sneg, accum_out=ps4[:, c:c + 1])
                else:
                    nc.scalar.activation(
                        u2[:, s], zm[:, s], AF.Square, bias=biasp, scale=sneg)
                    nc.scalar.activation(z_t[:, s], u2[:, s], AF.Exp, scale=-1.0,
                                         accum_out=ps4[:, c:c + 1])

            s_ps = psp.tile([BC, NCHUNK], FP)
            nc.tensor.matmul(s_ps, lhsT=e8c, rhs=ps4, start=True, stop=True)
            junk4 = const.tile([BC, NCHUNK], FP)
            tot16 = const.tile([BC, 1], FP)
            nc.scalar.activation(junk4, s_ps, AF.Identity,
                                 bias=0.0, scale=1.0, accum_out=tot16)
            s16 = const.tile([BC, 1], FP)
            nc.vector.tensor_scalar(s16, tot16, 1e-12, None, op0=ALU.max)
            inv16 = const.tile([BC, 1], FP)
            nc.vector.reciprocal(inv16, s16)
            invp_ps = psp.tile([128, 1], FP)
            nc.tensor.matmul(invp_ps, lhsT=e8tc, rhs=inv16, start=True, stop=True)
            invp = const.tile([128, 1], FP)
            nc.vector.tensor_copy(invp, invp_ps)

            zn = big.tile([128, CF], FP)
            attn_ap = attn[:].rearrange("b (j f) -> (b j) f", j=J)
            for c in range(NCHUNK):
                s = slice(c * CW, (c + 1) * CW)
                nc.vector.tensor_scalar(
                    zn[:, s], z_t[:, s], invp, None, op0=ALU.mult)
                nc.sync.dma_start(out=attn_ap[:, s], in_=zn[:, s])

    return nc


def _patch_bir(data):
    # This walrus build encodes at most one sync wait per instruction and
    # rejects the sequencer-only EVENT_SEMAPHORE_RANGE_CLEAR InstISA, both
    # emitted by the Tile teardown.  Split multi-wait drains into a chain
    # of single-wait drains and drop the range-clear (semaphores are reset
    # by the runtime between executions).
    for f in data["functions"]:
        for b in f["blocks"]:
            new_ins = []
            for ins in b["instructions"]:
                if (ins.get("opcode") == "ISA"
                        and ins.get("op_name") == "EVENT_SEMAPHORE_RANGE_CLEAR"):
                    continue
                si = ins.get("sync_info") or {}
                waits = si.get("on_wait") or []
                if len(waits) > 1:
                    for k, w in enumerate(waits[:-1]):
                        new_ins.append({
                            "debug": ins.get("debug", 0),
                            "engine": ins["engine"],
                            "ins": [], "outs": [],
                            "name": f"{ins['name']}w{k}",
                            "opcode": "Drain",
                            "sync_info": {"on_update": [], "on_wait": [w]},
                        })
                    ins = dict(ins)
                    ins["sync_info"] = {
                        "on_update": si.get("on_update") or [],
                        "on_wait": [waits[-1]],
                    }
                new_ins.append(ins)
            b["instructions"] = new_ins
    return data


_NC_CACHE = {}


def _get_nc():
    if "nc" not in _NC_CACHE:
        nc = _build_nc()
        orig = nc.to_json_bytes
        nc.to_json_bytes = lambda: orjson.dumps(
            _patch_bir(orjson.loads(orig())))
        _NC_CACHE["nc"] = nc
    return _NC_CACHE["nc"]


def make_in_maps(inputs):
    dec = np.asarray(inputs["decoder_outputs"], np.float32)      # (B,1,D)
    mu_old = np.asarray(inputs["mu_old"], np.float32).reshape(B)
    sg_old = np.asarray(inputs["sigma_old"], np.float32).reshape(B)
    mc_old = np.asarray(inputs["mean_content_old"], np.float32).reshape(B)
    ma_old = np.asarray(inputs["mean_attn_old"], np.float32).reshape(B)
    lens = np.asarray(inputs["source_lengths"], np.int32).reshape(B)
    step = int(np.asarray(inputs["step"]))
    conf_temp = float(np.asarray(inputs["conf_temp"]))
    conf_bias = float(np.asarray(inputs["conf_bias"]))

    w1cat = np.ascontiguousarray(
        np.concatenate([np.asarray(inputs["Wmu1"], np.float32),
                        np.asarray(inputs["Wsg1"], np.float32)], axis=1))
    w2blk = np.zeros((2 * H, 2 * NB), np.float32)
    w2blk[0:H, 0:NB] = np.asarray(inputs["Wmu2"], np.float32)
    w2blk[H:2 * H, NB:2 * NB] = np.asarray(inputs["Wsg2"], np.float32)
    b1cat = np.concatenate([np.asarray(inputs["bmu1"], np.float32),
                            np.asarray(inputs["bsg1"], np.float32)]).reshape(2 * H, 1)
    b2cat = np.concatenate([np.asarray(inputs["bmu2"], np.float32),
                            np.asarray(inputs["bsg2"], np.float32)]).reshape(2 * NB, 1)

    id16 = np.eye(BC, dtype=np.float32)
    b16 = (np.arange(BC, dtype=np.float32) * L).reshape(BC, 1)
    e8t = np.zeros((BC, 128), np.float32)
    for b_ in range(BC):
        e8t[b_, b_ * J:(b_ + 1) * J] = 1.0
    sm16 = np.ascontiguousarray(
        np.concatenate([id16, b16, e8t], axis=1))            # [16,145]
    p2048 = (np.arange(128, dtype=np.float32) * CF).reshape(128, 1)
    sm128 = np.ascontiguousarray(
        np.concatenate([e8t.T, p2048], axis=1))              # [128,17]
    fneg = np.ascontiguousarray(
        -np.tile(np.arange(1, CF + 1, dtype=np.float32), (128, 1)))

    in_maps = []
    for c in range(NCORES):
        r = slice(c * BC, (c + 1) * BC)
        sclc = np.zeros((BC, 8), np.float32)
        sclc[:, 0] = mu_old[r]
        sclc[:, 1] = sg_old[r]
        sclc[:, 2] = mc_old[r]
        sclc[:, 3] = ma_old[r]
        sclc[:, 4] = lens[r].astype(np.float32)
        sclc[:, 5] = step + 1.0
        sclc[:, 6] = -1.0 / conf_temp
        sclc[:, 7] = max(conf_bias, 0.0) / conf_temp
        in_maps.append({
            "decT": np.ascontiguousarray(dec[r, 0, :].T),
            "w1": w1cat, "b1": b1cat, "w2": w2blk, "b2": b2cat,
            "scl": sclc, "sm16": sm16, "sm128": sm128, "cst": fneg,
        })
    return in_maps


def kernel(**inputs):
    in_maps = make_in_maps(inputs)
    res = run_bass_kernel_spmd(_get_nc(), in_maps, list(range(NCORES)))
    results = res.results

    pos_attn = np.concatenate([results[c]["attn"] for c in range(NCORES)],
                              axis=0).reshape(B, 1, L)
    mu = np.concatenate([results[c]["mu_o"] for c in range(NCORES)],
                        axis=0).reshape(B, 1, 1)
    sigma = np.concatenate([results[c]["sg_o"] for c in range(NCORES)],
                           axis=0).reshape(B, 1, 1)
    conf = np.concatenate([results[c]["cf_o"] for c in range(NCORES)],
                          axis=0).reshape(B, 1, 1)
    return pos_attn, conf, mu, sigma
